# revision 8
# baseline (speedup 1.0000x reference)
"""EMA (exponential moving average) kernel for Trainium2, 8 NeuronCores.

Problem: y[b,c,f,t] = w*x[b,c,f,t] + (1-w)*y[b,c,f,t-1], y[...,-1] = initial_state.
Shapes: mag_spec [8,2,257,6000] f32, initial_state [8,2,257,1] f32, weights [1] f32.

Sharding: data-parallel over batch. Core i gets b=i -> [514, 6000] rows,
each row an independent scan along time.

Per core, per 128-row block: DMA-in the whole [128, 6000] row-block (split
into 4 stripes for DMA-queue parallelism) -> one ACT prescale (w*x) -> one
DVE tensor_tensor_scan over all 6000 columns (state = (1-w)*state + w*x,
the native first-order recurrence instruction, ~2 cycles/column) -> DMA-out.
One scan per block means no carry chaining and minimal cross-engine
handoffs (the scan was latency-bound, not throughput-bound, when chunked).
The 2 leftover rows (514 = 4*128 + 2) are one extra [2, 6000] scan.
"""

import numpy as np

B, C, F, T = 8, 2, 257, 6000
R = C * F  # 514 rows per core
P = 128  # partitions
N_CORES = 8
N_BLOCKS = R // P  # 4 full blocks; 2-row tail handled separately
TAIL = R - N_BLOCKS * P  # 2

# knobs for test harness
TRACE = False
LAST_EXEC_NS = None
LAST_RESULTS = None
BUFS_X = 3
BUFS_XW = 2
DMA_SPLIT = 4  # stripes per block DMA

_cache = {}


def _build_bass(w: float, a: float):
    import concourse.bacc as bacc
    import concourse.mybir as mybir
    from concourse.tile import TileContext

    # Bacc (not Bass): its finalize() runs generate_event_semaphores, which
    # splits sync waits to satisfy the per-instruction wait-slot limits
    # (DMA and the scan format only have 1-2 slots).
    nc = bacc.Bacc(None)
    x_d = nc.dram_tensor("x", [R, T], mybir.dt.float32, kind="ExternalInput")
    init_d = nc.dram_tensor("init", [R, 1], mybir.dt.float32, kind="ExternalInput")
    y_d = nc.dram_tensor("y", [R, T], mybir.dt.float32, kind="ExternalOutput")

    SP = P // DMA_SPLIT  # rows per DMA stripe

    with TileContext(nc) as tc:
        with (
            tc.tile_pool(name="const", bufs=1) as cpool,
            tc.tile_pool(name="xp", bufs=BUFS_X) as xpool,
            tc.tile_pool(name="wp", bufs=BUFS_XW) as wpool,
            tc.tile_pool(name="ip", bufs=N_BLOCKS + 1) as ipool,
        ):
            a_tile = cpool.tile([P, T], mybir.dt.float32)
            nc.vector.memset(a_tile[:], a)

            def emit_block(blk, rows):
                init_t = ipool.tile([P, 1], mybir.dt.float32, tag="init")
                nc.sync.dma_start(
                    out=init_t[:rows], in_=init_d[blk : blk + rows, :]
                )
                x_t = xpool.tile([P, T], mybir.dt.float32, tag="x")
                for s in range(0, rows, SP):
                    hi = min(s + SP, rows)
                    nc.sync.dma_start(
                        out=x_t[s:hi], in_=x_d[blk + s : blk + hi, :]
                    )
                xw_t = wpool.tile([P, T], mybir.dt.float32, tag="xw")
                nc.scalar.mul(xw_t[:rows], x_t[:rows], w)
                # scan in place over the ACT output (verified safe: the scan
                # writes column t strictly after reading it)
                nc.vector.tensor_tensor_scan(
                    out=xw_t[:rows],
                    data0=a_tile[:rows],
                    data1=xw_t[:rows],
                    initial=init_t[:rows, 0:1],
                    op0=mybir.AluOpType.mult,
                    op1=mybir.AluOpType.add,
                )
                for s in range(0, rows, SP):
                    hi = min(s + SP, rows)
                    nc.sync.dma_start(
                        out=y_d[blk + s : blk + hi, :], in_=xw_t[s:hi]
                    )

            for blk in range(0, N_BLOCKS * P, P):
                emit_block(blk, P)
            if TAIL:
                emit_block(N_BLOCKS * P, TAIL)
    nc.finalize()
    return nc


def kernel(mag_spec, initial_state, weights):
    global LAST_EXEC_NS, LAST_RESULTS
    from concourse.bass_utils import run_bass_kernel_spmd

    mag_spec = np.asarray(mag_spec, dtype=np.float32)
    initial_state = np.asarray(initial_state, dtype=np.float32)
    w = float(np.clip(np.asarray(weights, dtype=np.float32), 0.0, 1.0).reshape(-1)[0])
    a = float(np.float32(1.0) - np.float32(w))

    key = (w, a, BUFS_X, BUFS_XW, DMA_SPLIT)
    if key not in _cache:
        _cache[key] = _build_bass(w, a)
    nc = _cache[key]

    in_maps = []
    for i in range(N_CORES):
        in_maps.append(
            {
                "x": np.ascontiguousarray(mag_spec[i].reshape(R, T)),
                "init": np.ascontiguousarray(initial_state[i].reshape(R, 1)),
            }
        )

    res = run_bass_kernel_spmd(nc, in_maps, list(range(N_CORES)), trace=TRACE)
    LAST_EXEC_NS = res.exec_time_ns
    LAST_RESULTS = res
    out = np.stack(
        [res.results[i]["y"].reshape(C, F, T) for i in range(N_CORES)], axis=0
    )
    return out


# revision 11
# speedup vs baseline: 1.7893x; 1.7893x over previous
"""EMA (exponential moving average) kernel for Trainium2, 8 NeuronCores.

Problem: y[b,c,f,t] = w*x[b,c,f,t] + (1-w)*y[b,c,f,t-1], y[...,-1] = initial_state.
Shapes: mag_spec [8,2,257,6000] f32, initial_state [8,2,257,1] f32, weights [1] f32.

Sharding: data-parallel over batch. Core i gets b=i -> [514, 6000] rows,
each row an independent scan along time.

Per core, per 128-row block: DMA-in the whole [128, 6000] row-block (split
into 4 stripes for DMA-queue parallelism) -> one ACT prescale (w*x) -> one
DVE tensor_tensor_scan over all 6000 columns (state = (1-w)*state + w*x,
the native first-order recurrence instruction, ~2 cycles/column) -> DMA-out.
One scan per block means no carry chaining and minimal cross-engine
handoffs (the scan was latency-bound, not throughput-bound, when chunked).
The 2 leftover rows (514 = 4*128 + 2) are one extra [2, 6000] scan.
"""

import numpy as np

B, C, F, T = 8, 2, 257, 6000
R = C * F  # 514 rows per core
P = 128  # partitions
N_CORES = 8
N_BLOCKS = R // P  # 4 full blocks; 2-row tail handled separately
TAIL = R - N_BLOCKS * P  # 2

# knobs for test harness
TRACE = False
LAST_EXEC_NS = None
LAST_RESULTS = None
BUFS_X = 3
BUFS_XW = 2
DMA_SPLIT = 1  # stripes per block DMA; 128-partition DMAs hit all 16 SBUF ports
TAIL_GPSIMD = False  # scan opcode is not supported on Pool (walrus NCC_IXCG966)

_cache = {}


def _build_bass(w: float, a: float):
    import concourse.bacc as bacc
    import concourse.mybir as mybir
    from concourse.tile import TileContext

    # Bacc (not Bass): its finalize() runs generate_event_semaphores, which
    # splits sync waits to satisfy the per-instruction wait-slot limits
    # (DMA and the scan format only have 1-2 slots).
    nc = bacc.Bacc(None)
    x_d = nc.dram_tensor("x", [R, T], mybir.dt.float32, kind="ExternalInput")
    init_d = nc.dram_tensor("init", [R, 1], mybir.dt.float32, kind="ExternalInput")
    y_d = nc.dram_tensor("y", [R, T], mybir.dt.float32, kind="ExternalOutput")

    SP = P // DMA_SPLIT  # rows per DMA stripe

    with TileContext(nc) as tc:
        with (
            tc.tile_pool(name="const", bufs=1) as cpool,
            tc.tile_pool(name="xp", bufs=BUFS_X) as xpool,
            tc.tile_pool(name="wp", bufs=BUFS_XW) as wpool,
            tc.tile_pool(name="ip", bufs=N_BLOCKS + 1) as ipool,
        ):
            a_tile = cpool.tile([P, T], mybir.dt.float32)
            nc.gpsimd.memset(a_tile[:], a)

            def emit_block(blk, rows, scan_engine):
                init_t = ipool.tile([P, 1], mybir.dt.float32, tag="init")
                nc.sync.dma_start(
                    out=init_t[:rows], in_=init_d[blk : blk + rows, :]
                )
                x_t = xpool.tile([P, T], mybir.dt.float32, tag="x")
                for s in range(0, rows, SP):
                    hi = min(s + SP, rows)
                    nc.sync.dma_start(
                        out=x_t[s:hi], in_=x_d[blk + s : blk + hi, :]
                    )
                xw_t = wpool.tile([P, T], mybir.dt.float32, tag="xw")
                nc.scalar.mul(xw_t[:rows], x_t[:rows], w)
                # scan in place over the ACT output (verified safe: the scan
                # writes column t strictly after reading it)
                scan_engine.tensor_tensor_scan(
                    out=xw_t[:rows],
                    data0=a_tile[:rows],
                    data1=xw_t[:rows],
                    initial=init_t[:rows, 0:1],
                    op0=mybir.AluOpType.mult,
                    op1=mybir.AluOpType.add,
                )
                for s in range(0, rows, SP):
                    hi = min(s + SP, rows)
                    nc.sync.dma_start(
                        out=y_d[blk + s : blk + hi, :], in_=xw_t[s:hi]
                    )

            for blk in range(0, N_BLOCKS * P, P):
                emit_block(blk, P, nc.vector)
            if TAIL:
                emit_block(
                    N_BLOCKS * P, TAIL, nc.gpsimd if TAIL_GPSIMD else nc.vector
                )
    nc.finalize()
    return nc


def kernel(mag_spec, initial_state, weights):
    global LAST_EXEC_NS, LAST_RESULTS
    from concourse.bass_utils import run_bass_kernel_spmd

    mag_spec = np.asarray(mag_spec, dtype=np.float32)
    initial_state = np.asarray(initial_state, dtype=np.float32)
    w = float(np.clip(np.asarray(weights, dtype=np.float32), 0.0, 1.0).reshape(-1)[0])
    a = float(np.float32(1.0) - np.float32(w))

    key = (w, a, BUFS_X, BUFS_XW, DMA_SPLIT)
    if key not in _cache:
        _cache[key] = _build_bass(w, a)
    nc = _cache[key]

    in_maps = []
    for i in range(N_CORES):
        in_maps.append(
            {
                "x": np.ascontiguousarray(mag_spec[i].reshape(R, T)),
                "init": np.ascontiguousarray(initial_state[i].reshape(R, 1)),
            }
        )

    res = run_bass_kernel_spmd(nc, in_maps, list(range(N_CORES)), trace=TRACE)
    LAST_EXEC_NS = res.exec_time_ns
    LAST_RESULTS = res
    out = np.stack(
        [res.results[i]["y"].reshape(C, F, T) for i in range(N_CORES)], axis=0
    )
    return out


# revision 13
# speedup vs baseline: 1.9715x; 1.1018x over previous
"""EMA (exponential moving average) kernel for Trainium2, 8 NeuronCores.

Problem: y[b,c,f,t] = w*x[b,c,f,t] + (1-w)*y[b,c,f,t-1], y[...,-1] = initial_state.
Shapes: mag_spec [8,2,257,6000] f32, initial_state [8,2,257,1] f32, weights [1] f32.

Sharding: data-parallel over batch. Core i gets b=i -> [514, 6000] rows,
each row an independent scan along time.

Per core, per 128-row block: DMA-in the whole [128, 6000] row-block (split
into 4 stripes for DMA-queue parallelism) -> one ACT prescale (w*x) -> one
DVE tensor_tensor_scan over all 6000 columns (state = (1-w)*state + w*x,
the native first-order recurrence instruction, ~2 cycles/column) -> DMA-out.
One scan per block means no carry chaining and minimal cross-engine
handoffs (the scan was latency-bound, not throughput-bound, when chunked).
The 2 leftover rows (514 = 4*128 + 2) are one extra [2, 6000] scan.
"""

import numpy as np

B, C, F, T = 8, 2, 257, 6000
R = C * F  # 514 rows per core
P = 128  # partitions
N_CORES = 8
N_BLOCKS = R // P  # 4 full blocks; 2-row tail handled separately
TAIL = R - N_BLOCKS * P  # 2

# knobs for test harness
TRACE = False
LAST_EXEC_NS = None
LAST_RESULTS = None
BUFS_X = 3
BUFS_XW = 3
CH = 1500  # in-DMA / prescale chunk width (full 128-partition transfers)
TAIL_GPSIMD = False  # scan opcode is not supported on Pool (walrus NCC_IXCG966)

_cache = {}


def _build_bass(w: float, a: float):
    import concourse.bacc as bacc
    import concourse.mybir as mybir
    from concourse.tile import TileContext

    # Bacc (not Bass): its finalize() runs generate_event_semaphores, which
    # splits sync waits to satisfy the per-instruction wait-slot limits
    # (DMA and the scan format only have 1-2 slots).
    nc = bacc.Bacc(None)
    x_d = nc.dram_tensor("x", [R, T], mybir.dt.float32, kind="ExternalInput")
    init_d = nc.dram_tensor("init", [R, 1], mybir.dt.float32, kind="ExternalInput")
    y_d = nc.dram_tensor("y", [R, T], mybir.dt.float32, kind="ExternalOutput")

    with TileContext(nc) as tc:
        with (
            tc.tile_pool(name="const", bufs=1) as cpool,
            tc.tile_pool(name="xp", bufs=BUFS_X) as xpool,
            tc.tile_pool(name="wp", bufs=BUFS_XW) as wpool,
            tc.tile_pool(name="ip", bufs=N_BLOCKS + 1) as ipool,
        ):
            a_tile = cpool.tile([P, T], mybir.dt.float32)
            nc.gpsimd.memset(a_tile[:], a)

            def emit_block(blk, rows, scan_engine):
                init_t = ipool.tile([P, 1], mybir.dt.float32, tag="init")
                nc.sync.dma_start(
                    out=init_t[:rows], in_=init_d[blk : blk + rows, :]
                )
                # Chunk the in-DMA and prescale along time so ACT overlaps
                # the (serialized, ~300 GB/s) HWDGE transfers; the scan runs
                # once over the whole block. All DMAs keep 128 partitions
                # (16-SBUF-port rule).
                x_t = xpool.tile([P, T], mybir.dt.float32, tag="x")
                xw_t = wpool.tile([P, T], mybir.dt.float32, tag="xw")
                for lo in range(0, T, CH):
                    nc.sync.dma_start(
                        out=x_t[:rows, lo : lo + CH],
                        in_=x_d[blk : blk + rows, lo : lo + CH],
                    )
                    nc.scalar.mul(
                        xw_t[:rows, lo : lo + CH], x_t[:rows, lo : lo + CH], w
                    )
                # scan in place over the ACT output (verified safe: the scan
                # writes column t strictly after reading it)
                scan_engine.tensor_tensor_scan(
                    out=xw_t[:rows],
                    data0=a_tile[:rows],
                    data1=xw_t[:rows],
                    initial=init_t[:rows, 0:1],
                    op0=mybir.AluOpType.mult,
                    op1=mybir.AluOpType.add,
                )
                # out-DMA on the gpsimd SWDGE queue — separate from the
                # in-DMA HWDGE queue so in/out transfers run concurrently
                nc.gpsimd.dma_start(out=y_d[blk : blk + rows, :], in_=xw_t[:rows])

            for blk in range(0, N_BLOCKS * P, P):
                emit_block(blk, P, nc.vector)
            if TAIL:
                emit_block(
                    N_BLOCKS * P, TAIL, nc.gpsimd if TAIL_GPSIMD else nc.vector
                )
    nc.finalize()
    return nc


def kernel(mag_spec, initial_state, weights):
    global LAST_EXEC_NS, LAST_RESULTS
    from concourse.bass_utils import run_bass_kernel_spmd

    mag_spec = np.asarray(mag_spec, dtype=np.float32)
    initial_state = np.asarray(initial_state, dtype=np.float32)
    w = float(np.clip(np.asarray(weights, dtype=np.float32), 0.0, 1.0).reshape(-1)[0])
    a = float(np.float32(1.0) - np.float32(w))

    key = (w, a, BUFS_X, BUFS_XW, DMA_SPLIT)
    if key not in _cache:
        _cache[key] = _build_bass(w, a)
    nc = _cache[key]

    in_maps = []
    for i in range(N_CORES):
        in_maps.append(
            {
                "x": np.ascontiguousarray(mag_spec[i].reshape(R, T)),
                "init": np.ascontiguousarray(initial_state[i].reshape(R, 1)),
            }
        )

    res = run_bass_kernel_spmd(nc, in_maps, list(range(N_CORES)), trace=TRACE)
    LAST_EXEC_NS = res.exec_time_ns
    LAST_RESULTS = res
    out = np.stack(
        [res.results[i]["y"].reshape(C, F, T) for i in range(N_CORES)], axis=0
    )
    return out
